# revision 1
# baseline (speedup 1.0000x reference)
"""DiffSLIC kernel distributed over 8 NeuronCores.

Sharding: 8 shards = 2 (batch) x 4 (cluster-row groups of the 16x16
superpixel grid). Each shard receives a zero-padded pixel slab of
8 block-rows x 18 block-cols (128 x 288 pixels) covering its 6 local
cluster rows (4 owned + 1 halo each side) plus the 1-block pixel halo
every cluster's 48x48 window needs. Cluster iterations are
embarrassingly parallel given the halo (each cluster's softmax/update
touches only its own 3x3 block neighborhood), so the 5 iterations run
with zero communication; halo clusters are recomputed redundantly.

Key reformulation vs the reference: the (b,c,2304,16,16) unfold is never
materialized. Similarities are computed as 9 shifted block einsums, the
softmax runs over the stacked (9*256) axis, and the cluster update uses
the UNNORMALIZED exp weights (the softmax denominator cancels under the
subsequent l2-normalization). Out-of-image "fake" halo clusters stay
exactly zero through all iterations (masked weights sum to 0), which
reproduces the reference's zero-padded-candidate semantics for the final
pixel-to-superpixel softmax.
"""

import numpy as np

H = W = 256
C = 32
B = 2
HS = WS = 16          # superpixel grid
SH = SW = 16          # stride / block size
N_ITER = 5
TAU = 0.01
GROUPS = 4            # cluster-row groups (spatial shards per batch)
OWN = HS // GROUPS    # 4 owned cluster rows per shard

_compiled = None


def _shard_program(slab):
    """slab: (C, 128, 288) f32, zero-padded pixel slab.

    Block-row r of the slab = global block row 4g-2+r; block-col q =
    global block col q-1. Local cluster rows lr=0..5 = global 4g-1+lr,
    sitting at slab block rows lr+1; owned rows are lr=1..4.
    Returns (clst_out (C,4,16), s2p_out (2304,4,16), p2s_out (9,64,256)).
    """
    import jax.numpy as jnp

    inv_tau = 1.0 / TAU

    # raw blocks: (c, br, pi, bc, pj) -> (c, br, bc, pi, pj)
    raw = slab.reshape(C, 8, SH, 18, SW).transpose(0, 1, 3, 2, 4)

    # cluster init: block means of the RAW slab for block rows 1..6
    # (= local cluster rows 0..5), block cols 1..16 (= cluster cols 0..15)
    clst = raw[:, 1:7, 1:17].mean(axis=(3, 4))          # (C, 6, 16)

    def l2n(t, axis=0):
        n = jnp.sqrt((t * t).sum(axis=axis, keepdims=True))
        return jnp.where(n > 0, t / n, 0.0)

    xb = l2n(raw.reshape(C, -1)).reshape(raw.shape)      # normalize pixels
    clst = l2n(clst)

    # shifted pixel-block views for the 9-neighborhood of each cluster:
    # shift (di,dj) in {0,1,2}^2; cluster (lr,cj) <-> slab block
    # (lr+1, cj+1), neighbor block (lr+di, cj+dj).
    shifts = [
        xb[:, di:di + 6, dj:dj + 16].reshape(C, 6, 16, SH * SW)
        for di in range(3) for dj in range(3)
    ]                                                    # 9 x (C,6,16,256)
    xs = jnp.stack(shifts, axis=3)                       # (C,6,16,9,256)

    w = None
    for _ in range(N_ITER):
        # sims for all 9 shifted blocks at once: (6,16,9,256)
        s = jnp.einsum('crq,crqkp->rqkp', clst, xs)
        mask = s != 0.0
        smax = jnp.max(jnp.where(mask, s, -jnp.inf), axis=(2, 3),
                       keepdims=True)
        smax = jnp.where(jnp.isfinite(smax), smax, 0.0)
        w = jnp.exp((s - smax) * inv_tau) * mask         # unnormalized
        clst = l2n(jnp.einsum('rqkp,crqkp->crq', w, xs))

    # s2p output for owned cluster rows lr=1..4: normalize final weights
    w_own = w[1:5]                                       # (4,16,9,256)
    w_own = w_own / w_own.sum(axis=(2, 3), keepdims=True)
    # reference patch index p = (di*16+pi)*48 + dj*16+pj
    s2p = w_own.reshape(4, 16, 3, 3, SH, SW)
    s2p = s2p.transpose(2, 4, 3, 5, 0, 1).reshape(48 * 48, 4, 16)

    # final pixel->superpixel assignment for owned block rows (lr=1..4):
    # candidates = 3x3 neighborhood of final clusters (zero outside).
    clst_p = jnp.pad(clst, ((0, 0), (0, 0), (1, 1)))     # pad cluster cols
    cand = jnp.stack([clst_p[:, r:r + 4, q:q + 16]
                      for r in range(3) for q in range(3)], axis=1)
    # cand: (C, 9, 4, 16); owned pixels xo: (C, 4, 16, 256)
    xo = xb[:, 2:6, 1:17].reshape(C, 4, 16, SH * SW)
    s9 = jnp.einsum('ckrq,crqp->rqpk', cand, xo)         # (4,16,256,9)
    m9 = s9 != 0.0
    mx9 = jnp.max(jnp.where(m9, s9, -jnp.inf), axis=3, keepdims=True)
    e9 = jnp.exp((s9 - mx9) * inv_tau) * m9
    p9 = e9 / e9.sum(axis=3, keepdims=True)              # (4,16,256,9)
    # fold to (9, 64, 256): rows = r*16+pi, cols = q*16+pj
    p2s = p9.reshape(4, 16, SH, SW, 9).transpose(4, 0, 2, 1, 3)
    p2s = p2s.reshape(9, 64, 256)

    clst_out = clst[:, 1:5]                              # (C,4,16)
    return clst_out, s2p, p2s


def _get_compiled():
    global _compiled
    if _compiled is None:
        import jax
        devs = jax.devices()[:8]
        fn = jax.jit(_shard_program)
        _compiled = (jax, devs, fn)
    return _compiled


def _build_slabs(x):
    """x: (B,C,H,W) -> (8, C, 128, 288) zero-padded numpy slabs."""
    xp = np.zeros((B, C, H + 4 * SH, W + 2 * SW), dtype=np.float32)
    xp[:, :, 2 * SH:2 * SH + H, SW:SW + W] = x
    slabs = np.empty((B * GROUPS, C, 8 * SH, 18 * SW), dtype=np.float32)
    for bb in range(B):
        for g in range(GROUPS):
            r0 = 4 * g * SH          # padded row of global block row 4g-2
            slabs[bb * GROUPS + g] = xp[bb, :, r0:r0 + 8 * SH, :]
    return slabs


def kernel(x):
    x = np.asarray(x, dtype=np.float32)
    jax, devs, fn = _get_compiled()
    slabs = _build_slabs(x)

    futs = [fn(jax.device_put(slabs[i], devs[i % len(devs)]))
            for i in range(B * GROUPS)]

    clst = np.empty((B, C, HS, WS), dtype=np.float32)
    s2p = np.empty((B, 48 * 48, HS, WS), dtype=np.float32)
    p2s = np.empty((B, 9, H, W), dtype=np.float32)
    for i, (c_o, s_o, p_o) in enumerate(futs):
        bb, g = divmod(i, GROUPS)
        clst[bb, :, 4 * g:4 * g + 4, :] = np.asarray(c_o)
        s2p[bb, :, 4 * g:4 * g + 4, :] = np.asarray(s_o)
        p2s[bb, :, 64 * g:64 * g + 64, :] = np.asarray(p_o)
    return clst, p2s, s2p


# revision 3
# speedup vs baseline: 2.9894x; 2.9894x over previous
"""DiffSLIC kernel distributed over 8 NeuronCores.

Sharding: 8 shards = 2 (batch) x 4 (cluster-row groups of the 16x16
superpixel grid). Each shard receives a zero-padded pixel slab of
8 block-rows x 18 block-cols (128 x 288 pixels) covering its 6 local
cluster rows (4 owned + 1 halo each side) plus the 1-block pixel halo
every cluster's 48x48 window needs. Cluster iterations are
embarrassingly parallel given the halo (each cluster's softmax/update
touches only its own 3x3 block neighborhood), so the 5 iterations run
with zero communication; halo clusters are recomputed redundantly.

Key reformulation vs the reference: the (b,c,2304,16,16) unfold is never
materialized. Similarities are computed as 9 shifted block einsums, the
softmax runs over the stacked (9*256) axis, and the cluster update uses
the UNNORMALIZED exp weights (the softmax denominator cancels under the
subsequent l2-normalization). Out-of-image "fake" halo clusters stay
exactly zero through all iterations (masked weights sum to 0), which
reproduces the reference's zero-padded-candidate semantics for the final
pixel-to-superpixel softmax.
"""

import numpy as np

H = W = 256
C = 32
B = 2
HS = WS = 16          # superpixel grid
SH = SW = 16          # stride / block size
N_ITER = 5
TAU = 0.01
GROUPS = 4            # cluster-row groups (spatial shards per batch)
OWN = HS // GROUPS    # 4 owned cluster rows per shard

_compiled = None


def _shard_program(slab):
    """slab: (C, 128, 288) f32, zero-padded pixel slab.

    Block-row r of the slab = global block row 4g-2+r; block-col q =
    global block col q-1. Local cluster rows lr=0..5 = global 4g-1+lr,
    sitting at slab block rows lr+1; owned rows are lr=1..4.
    Returns (clst_out (C,4,16), s2p_out (2304,4,16), p2s_out (9,64,256)).
    """
    import jax.numpy as jnp

    inv_tau = 1.0 / TAU

    # raw blocks: (c, br, pi, bc, pj) -> (c, br, bc, pi, pj)
    raw = slab.reshape(C, 8, SH, 18, SW).transpose(0, 1, 3, 2, 4)

    # cluster init: block means of the RAW slab for block rows 1..6
    # (= local cluster rows 0..5), block cols 1..16 (= cluster cols 0..15)
    clst = raw[:, 1:7, 1:17].mean(axis=(3, 4))          # (C, 6, 16)

    def l2n(t, axis=0):
        n = jnp.sqrt((t * t).sum(axis=axis, keepdims=True))
        return jnp.where(n > 0, t / n, 0.0)

    xb = l2n(raw.reshape(C, -1)).reshape(raw.shape)      # normalize pixels
    clst = l2n(clst)

    # shifted pixel-block views for the 9-neighborhood of each cluster:
    # shift (di,dj) in {0,1,2}^2; cluster (lr,cj) <-> slab block
    # (lr+1, cj+1), neighbor block (lr+di, cj+dj).
    shifts = [
        xb[:, di:di + 6, dj:dj + 16].reshape(C, 6, 16, SH * SW)
        for di in range(3) for dj in range(3)
    ]                                                    # 9 x (C,6,16,256)
    xs = jnp.stack(shifts, axis=3)                       # (C,6,16,9,256)

    w = None
    for _ in range(N_ITER):
        # sims for all 9 shifted blocks at once: (6,16,9,256)
        s = jnp.einsum('crq,crqkp->rqkp', clst, xs)
        mask = s != 0.0
        smax = jnp.max(jnp.where(mask, s, -jnp.inf), axis=(2, 3),
                       keepdims=True)
        smax = jnp.where(jnp.isfinite(smax), smax, 0.0)
        w = jnp.exp((s - smax) * inv_tau) * mask         # unnormalized
        clst = l2n(jnp.einsum('rqkp,crqkp->crq', w, xs))

    # s2p output for owned cluster rows lr=1..4: normalize final weights
    w_own = w[1:5]                                       # (4,16,9,256)
    w_own = w_own / w_own.sum(axis=(2, 3), keepdims=True)
    # reference patch index p = (di*16+pi)*48 + dj*16+pj
    s2p = w_own.reshape(4, 16, 3, 3, SH, SW)
    s2p = s2p.transpose(2, 4, 3, 5, 0, 1).reshape(48 * 48, 4, 16)

    # final pixel->superpixel assignment for owned block rows (lr=1..4):
    # candidates = 3x3 neighborhood of final clusters (zero outside).
    clst_p = jnp.pad(clst, ((0, 0), (0, 0), (1, 1)))     # pad cluster cols
    cand = jnp.stack([clst_p[:, r:r + 4, q:q + 16]
                      for r in range(3) for q in range(3)], axis=1)
    # cand: (C, 9, 4, 16); owned pixels xo: (C, 4, 16, 256)
    xo = xb[:, 2:6, 1:17].reshape(C, 4, 16, SH * SW)
    s9 = jnp.einsum('ckrq,crqp->rqpk', cand, xo)         # (4,16,256,9)
    m9 = s9 != 0.0
    mx9 = jnp.max(jnp.where(m9, s9, -jnp.inf), axis=3, keepdims=True)
    e9 = jnp.exp((s9 - mx9) * inv_tau) * m9
    p9 = e9 / e9.sum(axis=3, keepdims=True)              # (4,16,256,9)
    # fold to (9, 64, 256): rows = r*16+pi, cols = q*16+pj
    p2s = p9.reshape(4, 16, SH, SW, 9).transpose(4, 0, 2, 1, 3)
    p2s = p2s.reshape(9, 64, 256)

    clst_out = clst[:, 1:5]                              # (C,4,16)
    return clst_out, s2p, p2s


def _get_compiled():
    global _compiled
    if _compiled is None:
        import jax
        try:
            jax.config.update("jax_compilation_cache_dir", "/tmp/jax_cc_cache")
            jax.config.update("jax_persistent_cache_min_compile_time_secs", 1)
            jax.config.update("jax_persistent_cache_min_entry_size_bytes", 0)
        except Exception:
            pass
        devs = jax.devices()[:8]
        pfn = None
        try:
            pfn = jax.pmap(_shard_program, devices=devs)
        except Exception:
            pfn = None
        fn = jax.jit(_shard_program)
        _compiled = (jax, devs, fn, pfn)
    return _compiled


def _build_slabs(x):
    """x: (B,C,H,W) -> (8, C, 128, 288) zero-padded numpy slabs."""
    xp = np.zeros((B, C, H + 4 * SH, W + 2 * SW), dtype=np.float32)
    xp[:, :, 2 * SH:2 * SH + H, SW:SW + W] = x
    slabs = np.empty((B * GROUPS, C, 8 * SH, 18 * SW), dtype=np.float32)
    for bb in range(B):
        for g in range(GROUPS):
            r0 = 4 * g * SH          # padded row of global block row 4g-2
            slabs[bb * GROUPS + g] = xp[bb, :, r0:r0 + 8 * SH, :]
    return slabs


def kernel(x):
    x = np.asarray(x, dtype=np.float32)
    jax, devs, fn, pfn = _get_compiled()
    slabs = _build_slabs(x)

    outs = None
    if pfn is not None:
        try:
            c_a, s_a, p_a = pfn(slabs)       # one dispatch over 8 cores
            outs = (np.asarray(c_a), np.asarray(s_a), np.asarray(p_a))
        except Exception:
            outs = None
    if outs is None:                          # per-device fallback
        futs = [fn(jax.device_put(slabs[i], devs[i % len(devs)]))
                for i in range(B * GROUPS)]
        outs = tuple(
            np.stack([np.asarray(f[j]) for f in futs]) for j in range(3))

    c_a, s_a, p_a = outs
    clst = np.empty((B, C, HS, WS), dtype=np.float32)
    s2p = np.empty((B, 48 * 48, HS, WS), dtype=np.float32)
    p2s = np.empty((B, 9, H, W), dtype=np.float32)
    for i in range(B * GROUPS):
        bb, g = divmod(i, GROUPS)
        clst[bb, :, 4 * g:4 * g + 4, :] = c_a[i]
        s2p[bb, :, 4 * g:4 * g + 4, :] = s_a[i]
        p2s[bb, :, 64 * g:64 * g + 64, :] = p_a[i]
    return clst, p2s, s2p


# revision 4
# speedup vs baseline: 4.1242x; 1.3796x over previous
"""DiffSLIC kernel distributed over 8 NeuronCores.

Sharding: 8 shards = 2 (batch) x 4 (cluster-row groups of the 16x16
superpixel grid). Each shard receives a zero-padded pixel slab of
8 block-rows x 18 block-cols (128 x 288 pixels) covering its 6 local
cluster rows (4 owned + 1 halo each side) plus the 1-block pixel halo
every cluster's 48x48 window needs. Cluster iterations are
embarrassingly parallel given the halo (each cluster's softmax/update
touches only its own 3x3 block neighborhood), so the 5 iterations run
with zero communication; halo clusters are recomputed redundantly.

Key reformulation vs the reference: the (b,c,2304,16,16) unfold is never
materialized. Similarities are computed as 9 shifted block einsums, the
softmax runs over the stacked (9*256) axis, and the cluster update uses
the UNNORMALIZED exp weights (the softmax denominator cancels under the
subsequent l2-normalization). Out-of-image "fake" halo clusters stay
exactly zero through all iterations (masked weights sum to 0), which
reproduces the reference's zero-padded-candidate semantics for the final
pixel-to-superpixel softmax.
"""

import numpy as np

H = W = 256
C = 32
B = 2
HS = WS = 16          # superpixel grid
SH = SW = 16          # stride / block size
N_ITER = 5
TAU = 0.01
GROUPS = 4            # cluster-row groups (spatial shards per batch)
OWN = HS // GROUPS    # 4 owned cluster rows per shard

_compiled = None


def _shard_program(slab):
    """slab: (C, 128, 288) f32, zero-padded pixel slab.

    Block-row r of the slab = global block row 4g-2+r; block-col q =
    global block col q-1. Local cluster rows lr=0..5 = global 4g-1+lr,
    sitting at slab block rows lr+1; owned rows are lr=1..4.
    Returns (clst_out (C,4,16), s2p_out (2304,4,16), p2s_out (9,64,256)).
    """
    import jax.numpy as jnp

    inv_tau = 1.0 / TAU

    # raw blocks: (c, br, pi, bc, pj) -> (c, br, bc, pi, pj)
    raw = slab.reshape(C, 8, SH, 18, SW).transpose(0, 1, 3, 2, 4)

    # cluster init: block means of the RAW slab for block rows 1..6
    # (= local cluster rows 0..5), block cols 1..16 (= cluster cols 0..15)
    clst = raw[:, 1:7, 1:17].mean(axis=(3, 4))          # (C, 6, 16)

    def l2n(t, axis=0):
        n = jnp.sqrt((t * t).sum(axis=axis, keepdims=True))
        return jnp.where(n > 0, t / n, 0.0)

    xb = l2n(raw.reshape(C, -1)).reshape(raw.shape)      # normalize pixels
    clst = l2n(clst)

    # shifted pixel-block views for the 9-neighborhood of each cluster:
    # shift (di,dj) in {0,1,2}^2; cluster (lr,cj) <-> slab block
    # (lr+1, cj+1), neighbor block (lr+di, cj+dj).
    shifts = [
        xb[:, di:di + 6, dj:dj + 16].reshape(C, 6, 16, SH * SW)
        for di in range(3) for dj in range(3)
    ]                                                    # 9 x (C,6,16,256)
    xs = jnp.stack(shifts, axis=3)                       # (C,6,16,9,256)

    w = None
    for _ in range(N_ITER):
        # sims for all 9 shifted blocks at once: (6,16,9,256)
        s = jnp.einsum('crq,crqkp->rqkp', clst, xs)
        mask = s != 0.0
        smax = jnp.max(jnp.where(mask, s, -jnp.inf), axis=(2, 3),
                       keepdims=True)
        smax = jnp.where(jnp.isfinite(smax), smax, 0.0)
        w = jnp.exp((s - smax) * inv_tau) * mask         # unnormalized
        clst = l2n(jnp.einsum('rqkp,crqkp->crq', w, xs))

    # s2p output for owned cluster rows lr=1..4: normalize final weights
    w_own = w[1:5]                                       # (4,16,9,256)
    w_own = w_own / w_own.sum(axis=(2, 3), keepdims=True)
    # reference patch index p = (di*16+pi)*48 + dj*16+pj
    s2p = w_own.reshape(4, 16, 3, 3, SH, SW)
    s2p = s2p.transpose(2, 4, 3, 5, 0, 1).reshape(48 * 48, 4, 16)

    # final pixel->superpixel assignment for owned block rows (lr=1..4):
    # candidates = 3x3 neighborhood of final clusters (zero outside).
    clst_p = jnp.pad(clst, ((0, 0), (0, 0), (1, 1)))     # pad cluster cols
    cand = jnp.stack([clst_p[:, r:r + 4, q:q + 16]
                      for r in range(3) for q in range(3)], axis=1)
    # cand: (C, 9, 4, 16); owned pixels xo: (C, 4, 16, 256)
    xo = xb[:, 2:6, 1:17].reshape(C, 4, 16, SH * SW)
    s9 = jnp.einsum('ckrq,crqp->rqpk', cand, xo)         # (4,16,256,9)
    m9 = s9 != 0.0
    mx9 = jnp.max(jnp.where(m9, s9, -jnp.inf), axis=3, keepdims=True)
    e9 = jnp.exp((s9 - mx9) * inv_tau) * m9
    p9 = e9 / e9.sum(axis=3, keepdims=True)              # (4,16,256,9)
    # fold to (9, 64, 256): rows = r*16+pi, cols = q*16+pj
    p2s = p9.reshape(4, 16, SH, SW, 9).transpose(4, 0, 2, 1, 3)
    p2s = p2s.reshape(9, 64, 256)

    clst_out = clst[:, 1:5]                              # (C,4,16)
    return clst_out, s2p, p2s


def _get_compiled():
    global _compiled
    if _compiled is None:
        import jax
        try:
            jax.config.update("jax_compilation_cache_dir", "/tmp/jax_cc_cache")
            jax.config.update("jax_persistent_cache_min_compile_time_secs", 1)
            jax.config.update("jax_persistent_cache_min_entry_size_bytes", 0)
        except Exception:
            pass
        devs = jax.devices()[:8]
        pfn = None
        try:
            pfn = jax.pmap(_shard_program, devices=devs)
        except Exception:
            pfn = None
        fn = jax.jit(_shard_program)
        _compiled = (jax, devs, fn, pfn)
    return _compiled


def _build_slabs(x):
    """x: (B,C,H,W) -> (8, C, 128, 288) zero-padded numpy slabs."""
    xp = np.zeros((B, C, H + 4 * SH, W + 2 * SW), dtype=np.float32)
    xp[:, :, 2 * SH:2 * SH + H, SW:SW + W] = x
    slabs = np.empty((B * GROUPS, C, 8 * SH, 18 * SW), dtype=np.float32)
    for bb in range(B):
        for g in range(GROUPS):
            r0 = 4 * g * SH          # padded row of global block row 4g-2
            slabs[bb * GROUPS + g] = xp[bb, :, r0:r0 + 8 * SH, :]
    return slabs


def kernel(x):
    x = np.asarray(x, dtype=np.float32)
    jax, devs, fn, pfn = _get_compiled()
    slabs = _build_slabs(x)

    outs = None
    if pfn is not None:
        try:
            o = pfn(slabs)                   # one dispatch over 8 cores
            outs = tuple(jax.device_get(o))  # batched tunnel fetch
        except Exception:
            outs = None
    if outs is None:                          # per-device fallback
        futs = [fn(jax.device_put(slabs[i], devs[i % len(devs)]))
                for i in range(B * GROUPS)]
        outs = tuple(
            np.stack([np.asarray(f[j]) for f in futs]) for j in range(3))

    c_a, s_a, p_a = outs
    clst = np.empty((B, C, HS, WS), dtype=np.float32)
    s2p = np.empty((B, 48 * 48, HS, WS), dtype=np.float32)
    p2s = np.empty((B, 9, H, W), dtype=np.float32)
    for i in range(B * GROUPS):
        bb, g = divmod(i, GROUPS)
        clst[bb, :, 4 * g:4 * g + 4, :] = c_a[i]
        s2p[bb, :, 4 * g:4 * g + 4, :] = s_a[i]
        p2s[bb, :, 64 * g:64 * g + 64, :] = p_a[i]
    return clst, p2s, s2p
